# revision 36
# baseline (speedup 1.0000x reference)
"""RGCN (basis-decomposed relational GCN) forward on 8 Trainium2 NeuronCores.

Strategy: shard by destination node. Host assigns nodes to 8-node blocks with
LPT bin-packing so every block carries ~250 in-edges (cap 256 = 2 chunks of
128), then blocks round-robin to cores — scatter work is uniform across cores
and blocks with ~2% padding. On device, per 128-node group: one 4096-row
batched dma_gather pulls source rows (bf16); one broadcast is_equal builds all
32 chunk one-hots; 64 M=64 matmuls scatter-add into a [128,1024]x2 fp32 PSUM
region (whole group, both IN-halves); the scalar engine copies PSUM->SBUF;
a second matmul stage contracts with the 8 relation weights + root term.
PE is software-pipelined: scatter(g) runs back-to-back with stage2(g-1).
No collectives: each core owns its output rows outright.
"""

import heapq
import os
import sys

import numpy as np
import ml_dtypes

for _p in ("/opt/trn_rl_repo", "/root/.axon_site/_ro/trn_rl_repo"):
    if os.path.isdir(_p) and _p not in sys.path:
        sys.path.append(_p)

import concourse.bacc as bacc
import concourse.tile as tile
from concourse import mybir
from concourse.bass_utils import run_bass_kernel_spmd

BF16 = ml_dtypes.bfloat16
N, E, IN, OUT, R = 20000, 640000, 256, 800, 8
NCORES = 8
NPAD = 2560                  # padded node slots per core
D = 8                        # nodes per block
SEGS = D * R                 # 64 segments per block
BLOCKS = NPAD // D           # 320 blocks per core
GROUPS = NPAD // 128         # 20 groups of 128 nodes
BPG = BLOCKS // GROUPS       # 16 blocks per group
NBLK = NCORES * BLOCKS       # 2560 global blocks
CHUNK = 128

_PROGRAM_CACHE = {}
LAST_RESULT = None           # test harness reads profiling info from here
LAST_PROGRAM = None          # bench harness re-runs this program for timing
LAST_IN_MAPS = None


def _build(chunks):
    """Compile the SPMD program for per-block chunk counts (same on all cores)."""
    dt = mybir.dt
    nc = bacc.Bacc("TRN2", target_bir_lowering=False, debug=False,
                   enable_asserts=False, num_devices=NCORES)
    TOT = sum(chunks) * CHUNK
    x_d = nc.dram_tensor("x", [N, IN], dt.bfloat16, kind="ExternalInput").ap()
    idxs_d = nc.dram_tensor("idxs", [128, TOT // 16], dt.int16, kind="ExternalInput").ap()
    segl_d = nc.dram_tensor("segl", [128, TOT // 128], dt.bfloat16, kind="ExternalInput").ap()
    xidx_d = nc.dram_tensor("xidx", [128, NPAD // 16], dt.int16, kind="ExternalInput").ap()
    w_d = nc.dram_tensor("w", [128, R * 2 * OUT], dt.bfloat16, kind="ExternalInput").ap()
    root_d = nc.dram_tensor("root", [128, 2 * OUT], dt.bfloat16, kind="ExternalInput").ap()
    out_d = nc.dram_tensor("out", [NPAD, OUT], dt.bfloat16, kind="ExternalOutput").ap()

    # per-group chunk counts and slot bases
    gch = [sum(chunks[g * BPG:(g + 1) * BPG]) for g in range(GROUPS)]
    gbase = np.concatenate([[0], np.cumsum(gch)]).astype(int)
    # chunk index -> block-in-group, plus per-(group,block) first chunk
    blk_of_chunk = []        # per group: list of (blk, is_first, is_last)
    for g in range(GROUPS):
        lst = []
        for blk in range(BPG):
            cb = chunks[g * BPG + blk]
            for k in range(cb):
                lst.append((blk, k == 0, k == cb - 1))
        blk_of_chunk.append(lst)

    with tile.TileContext(nc) as tc:
        with tc.tile_pool(name="const", bufs=1) as cp, \
             tc.tile_pool(name="xgp", bufs=2) as xgp, \
             tc.tile_pool(name="xtp", bufs=2) as xtp, \
             tc.tile_pool(name="ohp", bufs=2) as ohp, \
             tc.tile_pool(name="stp", bufs=2) as stp, \
             tc.tile_pool(name="outp", bufs=2) as outp, \
             tc.tile_pool(name="psp", bufs=1, space="PSUM") as psp, \
             tc.tile_pool(name="pso", bufs=2, space="PSUM") as pso:
            # consts stream in demand order: group-0 idx slice first so the
            # gathers start immediately, then later idx slices and the two w
            # halves interleaved (stage2(0) consumes w relation-by-relation).
            c04 = gbase[min(5, GROUPS)] * 8
            idxs_sb = cp.tile([128, TOT // 16], dt.int16)
            nc.sync.dma_start(idxs_sb[:, :16], idxs_d[:, :16])
            segl_sb = cp.tile([128, TOT // 128], dt.bfloat16)
            nc.sync.dma_start(segl_sb[:, :gch[0]], segl_d[:, :gch[0]])
            nc.sync.dma_start(idxs_sb[:, 16:gch[0] * 8], idxs_d[:, 16:gch[0] * 8])
            nc.sync.dma_start(segl_sb[:, gch[0]:], segl_d[:, gch[0]:])
            xidx_sb = cp.tile([128, NPAD // 16], dt.int16)
            nc.sync.dma_start(xidx_sb[:], xidx_d[:, :])
            w_sb = cp.tile([128, R * 2 * OUT], dt.bfloat16)
            root_sb = cp.tile([128, 2 * OUT], dt.bfloat16)
            nc.sync.dma_start(idxs_sb[:, gch[0] * 8:c04], idxs_d[:, gch[0] * 8:c04])
            w4_sb = w_sb[:].rearrange("p (r h o) -> p r h o", h=2, o=OUT)
            w4_d = w_d[:, :].rearrange("p (r h o) -> p r h o", h=2, o=OUT)
            nc.sync.dma_start(w4_sb[:, :, 0, :], w4_d[:, :, 0, :])
            nc.sync.dma_start(root_sb[:], root_d[:, :])
            nc.sync.dma_start(w4_sb[:, :, 1, :], w4_d[:, :, 1, :])
            nc.sync.dma_start(idxs_sb[:, c04:], idxs_d[:, c04:])
            iota_i = cp.tile([128, SEGS], dt.int32)
            nc.gpsimd.iota(iota_i[:], pattern=[[1, SEGS]], base=0, channel_multiplier=0)
            iota_b = cp.tile([128, SEGS], dt.bfloat16)
            nc.vector.tensor_copy(iota_b[:], iota_i[:])

            def gather_group(g):
                gc = gch[g]
                # SWDGE descriptor ring caps one gather at 1024 rows (8 chunks).
                # group 0: fine-grained calls so the first scatter matmuls
                # start as early as possible
                steps = ([2] * 4 + [4, 4, 8, 8] + [8] * 100) if g == 0 else [8] * 100
                xg = xgp.tile([128, gc * IN], dt.bfloat16, tag="xg")
                c0 = 0
                for step in steps:
                    if c0 >= gc:
                        break
                    cn = min(step, gc - c0)
                    nc.gpsimd.dma_gather(
                        xg[:, c0 * IN:(c0 + cn) * IN].rearrange("p (c e) -> p c e", e=IN),
                        x_d[:, :],
                        idxs_sb[:, (gbase[g] + c0) * 8:(gbase[g] + c0 + cn) * 8],
                        cn * CHUNK, cn * CHUNK, IN)
                    c0 += cn
                # own-node rows, transposed: xTg[p, h, i] = x[own[g*128+i]][h*128+p]
                xTg = xtp.tile([128, 2 * 128], dt.bfloat16, tag="xT")
                nc.gpsimd.dma_gather(
                    xTg[:].rearrange("p (h i) -> p h i", i=128),
                    x_d[:, :], xidx_sb[:, g * 8:(g + 1) * 8],
                    128, 128, IN, transpose=True)
                return xg, xTg

            def onehot_group(g):
                gc = gch[g]
                oh = ohp.tile([128, gc * SEGS], dt.bfloat16, tag="oh")
                oh3 = oh[:].rearrange("p (c m) -> p c m", m=SEGS)
                # group 0 in pieces so the first scatter matmuls start early
                pieces = [4, 4, 8, 100] if g == 0 else [100]
                c0 = 0
                for step in pieces:
                    if c0 >= gc:
                        break
                    cn = min(step, gc - c0)
                    in0 = iota_b[:].unsqueeze(1).broadcast_to([128, cn, SEGS])
                    in1 = segl_sb[:, gbase[g] + c0:gbase[g] + c0 + cn].unsqueeze(2) \
                        .broadcast_to([128, cn, SEGS])
                    nc.vector.tensor_tensor(out=oh3[:, c0:c0 + cn, :], in0=in0,
                                            in1=in1, op=mybir.AluOpType.is_equal)
                    c0 += cn
                return oh

            def scatter_group(g, xg, oh):
                ps0 = psp.tile([128, BPG * SEGS], dt.float32, tag="ps0", name="ps0")
                ps1 = psp.tile([128, BPG * SEGS], dt.float32, tag="ps1", name="ps1")
                pss = [ps0, ps1]
                for ci, (blk, first, last) in enumerate(blk_of_chunk[g]):
                    for h in range(2):
                        nc.tensor.matmul(
                            out=pss[h][:, blk * SEGS:(blk + 1) * SEGS],
                            lhsT=xg[:, ci * IN + h * 128: ci * IN + (h + 1) * 128],
                            rhs=oh[:, ci * SEGS:(ci + 1) * SEGS],
                            start=first, stop=last)
                return pss

            def stcopy_group(pss):
                # st columns: h*1024 + r*128 + node(=blk*8+j) — the copy
                # permutes from the psum layout blk*64 + r*8 + j so stage-2
                # lhsT slices are contiguous (walrus allows only one free dim
                # on matmul stationary APs)
                st = stp.tile([128, 2 * BPG * SEGS], dt.bfloat16, tag="st")
                for h in range(2):
                    dst = st[:, h * 1024:(h + 1) * 1024].rearrange(
                        "p (r blk j) -> p r blk j", r=R, blk=BPG, j=D)
                    src = pss[h][:].rearrange(
                        "p (blk r j) -> p r blk j", blk=BPG, r=R, j=D)
                    nc.scalar.copy(dst, src)
                return st

            def stage2_group(g, st, xTg):
                # separate PSUM tiles per output bank so each copy depends
                # only on its own accumulation chain
                ops_a = pso.tile([128, 512], dt.float32, tag="opsa", name="ops_a")
                ops_b = pso.tile([128, 288], dt.float32, tag="opsb", name="ops_b")
                out_sb = outp.tile([128, OUT], dt.bfloat16, tag="osb")
                # h-major (first chains only need st half 0); fo innermost so
                # both output slices stream under one stationary load
                regions = ((ops_a, 0, 512), (ops_b, 512, 288))
                for h in range(2):
                    for src in range(R + 1):
                        if src == 0:
                            lhsT = xTg[:, h * 128:(h + 1) * 128]
                        else:
                            r8 = src - 1
                            lhsT = st[:, h * 1024 + r8 * 128: h * 1024 + (r8 + 1) * 128]
                        for (ops, fo, fl) in regions:
                            if src == 0:
                                rhs = root_sb[:, h * OUT + fo: h * OUT + fo + fl]
                            else:
                                rhs = w_sb[:, ((src - 1) * 2 + h) * OUT + fo:
                                           ((src - 1) * 2 + h) * OUT + fo + fl]
                            nc.tensor.matmul(
                                out=ops[:], lhsT=lhsT, rhs=rhs,
                                start=(h == 0 and src == 0),
                                stop=(h == 1 and src == R))
                for (ops, fo, fl) in regions:
                    nc.vector.tensor_copy(out_sb[:, fo:fo + fl], ops[:])
                    nc.sync.dma_start(out_d[g * 128:(g + 1) * 128, fo:fo + fl],
                                      out_sb[:, fo:fo + fl])

            # software pipeline: PE does scatter(g) then stage2(g-1)
            prev = None          # (st, xTg) of g-1
            for g in range(GROUPS + 1):
                if g < GROUPS:
                    xg, xTg = gather_group(g)
                    oh = onehot_group(g)
                    pss = scatter_group(g, xg, oh)
                    st = stcopy_group(pss)
                if prev is not None:
                    stage2_group(g - 1, *prev)
                prev = (st, xTg) if g < GROUPS else None
    nc.compile()
    return nc


def _balance(deg):
    """LPT-pack nodes into NBLK blocks of <=D nodes, balancing in-edge sums.

    Returns assign [NBLK, D] int32 (node id or -1)."""
    order = np.argsort(-deg, kind="stable")
    heap = [(0, gi) for gi in range(NBLK)]
    heapq.heapify(heap)
    counts = np.zeros(NBLK, np.int64)
    sums = np.zeros(NBLK, np.int64)
    assign = np.full((NBLK, D), -1, np.int32)
    for v in order:
        s, gi = heapq.heappop(heap)
        assign[gi, counts[gi]] = v
        sums[gi] += deg[v]
        counts[gi] += 1
        if counts[gi] < D:
            heapq.heappush(heap, (int(sums[gi]), gi))
    return assign


def kernel(x, edge_index, edge_type, bases, att, root, bias):
    global LAST_RESULT, LAST_PROGRAM, LAST_IN_MAPS
    x = np.asarray(x, dtype=np.float32)
    edge_index = np.asarray(edge_index, dtype=np.int32)
    edge_type = np.asarray(edge_type, dtype=np.int32)
    bases = np.asarray(bases, dtype=np.float32)
    att = np.asarray(att, dtype=np.float32)
    root = np.asarray(root, dtype=np.float32)
    bias = np.asarray(bias, dtype=np.float32)

    src_all, dst_all = edge_index[0], edge_index[1]
    deg = np.bincount(dst_all, minlength=N).astype(np.int64)
    assign = _balance(deg)                    # [NBLK, D] node ids
    # global block gi -> (core gi % 8, per-core block gi // 8)
    # node -> (core, padded slot p = block*D + j)
    node_core = np.full(N, -1, np.int32)
    node_slot = np.full(N, -1, np.int32)
    gi_idx, j_idx = np.nonzero(assign >= 0)
    nodes = assign[gi_idx, j_idx]
    node_core[nodes] = (gi_idx % NCORES).astype(np.int32)
    node_slot[nodes] = ((gi_idx // NCORES) * D + j_idx).astype(np.int32)

    core_of = node_core[dst_all]
    slot_of = node_slot[dst_all]

    # per-(core, block) edge counts -> shared chunk schedule
    blk_of = slot_of // D
    counts_cb = np.zeros((NCORES, BLOCKS), np.int64)
    np.add.at(counts_cb, (core_of, blk_of), 1)
    chunks = np.maximum(1, -(-counts_cb // CHUNK)).max(0)
    chunks = tuple(int(v) for v in chunks)
    TOT = sum(chunks) * CHUNK
    block_slot_start = np.concatenate([[0], np.cumsum(np.asarray(chunks) * CHUNK)])

    key = (chunks,)
    if key not in _PROGRAM_CACHE:
        _PROGRAM_CACHE[key] = _build(chunks)
    nc = _PROGRAM_CACHE[key]

    # shared weights
    W = np.einsum("rb,bio->rio", att, bases).astype(np.float32)      # [R, IN, OUT]
    w_dev = np.ascontiguousarray(
        W.reshape(R, 2, 128, OUT).transpose(2, 0, 1, 3).reshape(128, R * 2 * OUT)
    ).astype(BF16)
    root_dev = np.ascontiguousarray(
        root.reshape(2, 128, OUT).transpose(1, 0, 2).reshape(128, 2 * OUT)
    ).astype(BF16)
    x_bf = np.ascontiguousarray(x).astype(BF16)

    in_maps = []
    for c in range(NCORES):
        m = core_of == c
        src, slot, et = src_all[m], slot_of[m], edge_type[m]
        blk = slot // D
        j = slot % D
        seg = et * D + j                      # r-major within block
        order = np.argsort(blk, kind="stable")
        blk_s, seg_s, src_s = blk[order], seg[order], src[order]
        cnts = counts_cb[c]
        edge_block_start = np.concatenate([[0], np.cumsum(cnts)])
        within = np.arange(len(src_s)) - edge_block_start[blk_s]
        pos = block_slot_start[blk_s] + within

        src_slots = np.zeros(TOT, np.int16)
        segl_slots = np.full(TOT, -1.0, np.float32)
        src_slots[pos] = src_s.astype(np.int16)
        segl_slots[pos] = seg_s.astype(np.float32)

        idxs_dev = np.ascontiguousarray(np.tile(src_slots.reshape(-1, 16).T, (8, 1)))
        segl_dev = np.ascontiguousarray(
            segl_slots.reshape(-1, 128).T.astype(BF16))

        # own-node ids for the root-term transpose-gather (0 for empty slots;
        # those output rows are garbage and never read back)
        gis = np.arange(BLOCKS) * NCORES + c
        nodes_c = assign[gis].reshape(-1)                 # [NPAD] node or -1
        xidx = np.where(nodes_c >= 0, nodes_c, 0).astype(np.int16)
        xidx_dev = np.ascontiguousarray(np.tile(xidx.reshape(-1, 16).T, (8, 1)))

        in_maps.append({
            "x": x_bf, "idxs": idxs_dev, "segl": segl_dev,
            "xidx": xidx_dev, "w": w_dev, "root": root_dev,
        })

    LAST_PROGRAM, LAST_IN_MAPS = nc, in_maps
    res = run_bass_kernel_spmd(nc, in_maps, core_ids=list(range(NCORES)))
    LAST_RESULT = res

    out = np.empty((N, OUT), np.float32)
    for c in range(NCORES):
        gis = np.arange(BLOCKS) * NCORES + c
        nodes_c = assign[gis].reshape(-1)
        valid = nodes_c >= 0
        out[nodes_c[valid]] = res.results[c]["out"][valid].astype(np.float32)
    out += bias[None, :]
    return out
